# revision 46
# baseline (speedup 1.0000x reference)
"""Trainium2 Bass kernel for a 2-layer GCN (GCNConv x2 + linear head +
log_softmax) distributed across 8 NeuronCores.

Self-contained: accepts FULL inputs, shards internally, runs one SPMD Bass
program on cores 0-7 via run_bass_kernel_spmd, gathers the full output.

v2 strategy (node-sharded, halo-gather aggregation):
- Nodes sharded across 8 cores (12500/core, padded to NSH=12544 = 98 windows
  of 128). Per layer the node transform t = dinv * (h @ W) is written as bf16
  table rows padded to 128 elems (256B) — the dma_gather element floor.
- The shard is split into 4 window-aligned QUARTERS; quadrant q of the global
  table = quarter q of every core. Each layer runs 4 chunked AllGathers (one
  per quarter) so aggregation on quadrant q overlaps the collective for q+1.
- Self-loops are NOT materialized as edges: the local table rows are added at
  PSUM evacuation (out = relu(dinv*(psum + t_local) + bias)), saving ~12% of
  gather descriptors, S-matrix work, and matmuls.
- Aggregation: per (window-group, quadrant) granule one dma_gather (int16 idx
  relative to the quadrant table) fetches message rows; a 0/1 selection matrix
  S (DVE is_equal vs iota) scatter-adds each 128-edge chunk into the window's
  PSUM via TensorE: psum += S^T @ msgs. norm = dinv[src]*dinv[dst] factored:
  dinv[src] into table rows, dinv[dst] at evacuation.
- h1, t1, t2 and the dst-slot stream (dstl) live in SBUF; the output head is
  fused into layer-2 evacuation. No DRAM round-trips between stages.

Timing passes (repeats>1) are chained through a carry scaled by the
runtime-zero input `zmul` so no pass can be dead-code-eliminated.
"""
import numpy as np
import ml_dtypes

import concourse.bass as bass
import concourse.tile as tile
from concourse import bacc, mybir

P = 128
NCORES = 8
NQ = 4

BF16 = ml_dtypes.bfloat16

# gather tuning knobs
CFG = dict(queues=1, sp=False)


def ru(x, m):
    return (x + m - 1) // m * m


def preprocess(x, edge_index, W1, b1, W2, b2, W3, b3):
    """Host-side sharding/preprocessing. Returns (meta, in_maps)."""
    x = np.asarray(x, dtype=np.float32)
    ei = np.asarray(edge_index)
    N, IN_DIM = x.shape
    HID = np.asarray(W1).shape[1]
    OUT = np.asarray(W3).shape[1]
    assert N % NCORES == 0
    SH = N // NCORES            # real nodes per shard
    NSH = ru(SH, P)             # padded shard
    NW = NSH // P               # windows per core (98)
    QWIN = [25, 25, 24, 24]     # windows per quarter (sum = NW)
    assert sum(QWIN) == NW
    QW0 = np.cumsum([0] + QWIN)[:4]          # window offset per quarter
    QC = [w * P for w in QWIN]               # rows per core-quarter
    QROWS = [NCORES * c for c in QC]         # quadrant table rows
    assert max(QROWS) < 32768
    WQ = np.zeros(NW, dtype=np.int64)        # window -> quarter
    for q in range(NQ):
        WQ[QW0[q]:QW0[q] + QWIN[q]] = q

    SWG = []                    # window groups (start, count); one PSUM bank
    w0 = 0                      # per window, 6 banks for aggregation
    while w0 < NW:
        nw = min(6, NW - w0)
        SWG.append((w0, nw))
        w0 += nw

    src = np.asarray(ei[0], dtype=np.int64)
    dst = np.asarray(ei[1], dtype=np.int64)

    # degree includes the (virtual) self loop
    deg = (np.bincount(dst, minlength=N) + 1).astype(np.float32)
    dinv = (1.0 / np.sqrt(deg)).astype(np.float32)

    csrc = src // SH
    sl = src - csrc * SH                      # local src id in its shard
    swin = sl // P
    q = WQ[swin]                              # src quadrant (by quarter)
    rel = (csrc * np.asarray(QC)[q] + (sl - QW0[q] * P)).astype(np.int16)

    cdst = dst // SH
    dl = dst - cdst * SH
    w = dl // P                               # dst window
    slot = dl % P

    # per (core, quadrant, window) counts -> shared caps
    key = (cdst * NQ + q) * NW + w
    counts = np.bincount(key, minlength=NCORES * NQ * NW).reshape(NCORES, NQ, NW)
    caps = ru(counts.max(axis=0), P)          # [NQ, NW]

    # stream layout: window-group major, then quadrant, then window
    qw_off = np.zeros((NQ, NW), dtype=np.int64)
    granules = {}                             # (gw0, q) -> dict
    off = 0
    for (gw0, gnw) in SWG:
        for qq in range(NQ):
            wcaps = [int(caps[qq, gw0 + i]) for i in range(gnw)]
            n_idx = int(sum(wcaps))
            granules[(gw0, qq)] = dict(q=qq, w0=gw0, nw=gnw, off=off,
                                       n_idx=n_idx,
                                       wchunks=[c // P for c in wcaps])
            for i in range(gnw):
                qw_off[qq, gw0 + i] = off
                off += wcaps[i]
    TOT = off
    TOTC = TOT // P

    meta = dict(N=N, IN_DIM=IN_DIM, HID=HID, OUT=OUT, SH=SH, NSH=NSH, NW=NW,
                QWIN=QWIN, QW0=list(QW0), QC=QC, QROWS=QROWS, WQ=WQ.tolist(),
                SWG=SWG, granules=granules, TOT=TOT, TOTC=TOTC)

    iota_np = np.broadcast_to(np.arange(P, dtype=np.float32)[None, :], (P, P))
    iota_bf = np.ascontiguousarray(iota_np.astype(BF16))
    ident = np.eye(P, dtype=np.float32)
    W1b = np.asarray(W1, np.float32).astype(BF16)
    W2f = np.asarray(W2, np.float32)
    W3f = np.asarray(W3, np.float32)
    b1r = np.broadcast_to(np.asarray(b1, np.float32)[None, :], (P, HID)).copy()
    b2r = np.broadcast_to(np.asarray(b2, np.float32)[None, :], (P, HID)).copy()
    b3r = np.broadcast_to(np.asarray(b3, np.float32)[None, :], (P, OUT)).copy()

    # rank of each edge inside its (core, q, w) segment
    order = np.lexsort((w, q, cdst))
    q_s, w_s, slot_s, rel_s, cdst_s = (
        q[order], w[order], slot[order], rel[order], cdst[order])

    in_maps = []
    for c in range(NCORES):
        m = cdst_s == c
        e_rel = rel_s[m]
        e_slot = slot_s[m]
        e_q = q_s[m]
        e_w = w_s[m]
        seg_key = e_q * NW + e_w
        ne = len(seg_key)
        rank = np.zeros(ne, dtype=np.int64)
        if ne:
            seg_change = np.empty(ne, dtype=bool)
            seg_change[0] = True
            seg_change[1:] = seg_key[1:] != seg_key[:-1]
            seg_start = np.flatnonzero(seg_change)
            seg_len = np.diff(np.append(seg_start, ne))
            ar = np.arange(ne)
            rank = ar - np.repeat(ar[seg_start], seg_len)
        pos = qw_off[e_q, e_w] + rank

        rel16 = np.zeros(TOT, dtype=np.int16)
        slot_arr = np.full(TOT, 200.0, dtype=np.float32)
        rel16[pos] = e_rel
        slot_arr[pos] = e_slot.astype(np.float32)

        idx_wr = rel16.reshape(TOT // 16, 16).T          # [16, TOT/16]
        idx_rep = np.ascontiguousarray(np.tile(idx_wr, (8, 1)))  # [128, TOT/16]
        dstl = np.ascontiguousarray(
            slot_arr.reshape(TOTC, P).T.astype(BF16))    # [128, TOTC]

        xs = x[c * SH:(c + 1) * SH]                      # [SH, IN_DIM]
        x_t = np.zeros((IN_DIM, NSH), dtype=np.float32)
        x_t[:, :SH] = xs.T

        dv = np.zeros(NSH, dtype=np.float32)
        dv[:SH] = dinv[c * SH:(c + 1) * SH]
        dinv_t = np.ascontiguousarray(dv.reshape(NW, P).T)  # [128, NW]

        in_maps.append(dict(
            x_t=np.ascontiguousarray(x_t.astype(BF16)), dinv_t=dinv_t,
            idx_rep=idx_rep, dstl=dstl,
            w1=W1b, w2=W2f, w3=W3f, b1r=b1r, b2r=b2r, b3r=b3r,
            iota=iota_bf, ident=ident,
            zmul=np.zeros((P, 1), dtype=np.float32),
        ))
    return meta, in_maps


def build_program(meta, repeats=1, rep_stages=("t1", "ag1", "agg1", "t2", "ag2", "agg2")):
    IN_DIM, HID, OUT = meta["IN_DIM"], meta["HID"], meta["OUT"]
    NSH, NW = meta["NSH"], meta["NW"]
    QWIN, QW0, QC, QROWS, WQ = (meta["QWIN"], meta["QW0"], meta["QC"],
                                meta["QROWS"], meta["WQ"])
    SWG, granules, TOT, TOTC = meta["SWG"], meta["granules"], meta["TOT"], meta["TOTC"]
    KT = IN_DIM // P
    AF = mybir.ActivationFunctionType

    nc = bacc.Bacc("TRN2", target_bir_lowering=False, debug=False, num_devices=NCORES)
    f32, bf16, i16 = mybir.dt.float32, mybir.dt.bfloat16, mybir.dt.int16

    x_t = nc.dram_tensor("x_t", [IN_DIM, NSH], bf16, kind="ExternalInput")
    dinv_in = nc.dram_tensor("dinv_t", [P, NW], f32, kind="ExternalInput")
    idx_in = nc.dram_tensor("idx_rep", [P, TOT // 16], i16, kind="ExternalInput")
    dstl_in = nc.dram_tensor("dstl", [P, TOTC], bf16, kind="ExternalInput")
    w1_in = nc.dram_tensor("w1", [IN_DIM, HID], bf16, kind="ExternalInput")
    w2_in = nc.dram_tensor("w2", [HID, HID], f32, kind="ExternalInput")
    w3_in = nc.dram_tensor("w3", [HID, OUT], f32, kind="ExternalInput")
    b1_in = nc.dram_tensor("b1r", [P, HID], f32, kind="ExternalInput")
    b2_in = nc.dram_tensor("b2r", [P, HID], f32, kind="ExternalInput")
    b3_in = nc.dram_tensor("b3r", [P, OUT], f32, kind="ExternalInput")
    iota_in = nc.dram_tensor("iota", [P, P], bf16, kind="ExternalInput")
    id_in = nc.dram_tensor("ident", [P, P], f32, kind="ExternalInput")
    zmul_in = nc.dram_tensor("zmul", [P, 1], f32, kind="ExternalInput")
    out_ext = nc.dram_tensor("out", [NSH, OUT], f32, kind="ExternalOutput")

    # last window of each quarter (fires that quarter's collective)
    q_last_w = [QW0[q] + QWIN[q] - 1 for q in range(NQ)]

    with tile.TileContext(nc) as tc:
        with (
            tc.tile_pool(name="const", bufs=1) as cpool,
            tc.tile_pool(name="xload", bufs=3) as xpool,
            tc.tile_pool(name="tt", bufs=3) as ttpool,
            tc.tile_pool(name="idx", bufs=3) as ixpool,
            tc.tile_pool(name="gat", bufs=4) as gpool,
            tc.tile_pool(name="sel", bufs=3) as spool,
            tc.tile_pool(name="ev", bufs=2) as evpool,
            tc.tile_pool(name="hload", bufs=3) as hpool,
            tc.tile_pool(name="pagg", bufs=6, space="PSUM") as pagg,
            tc.tile_pool(name="pscr", bufs=2, space="PSUM") as pscr,
            tc.tile_pool(name="dram", bufs=1, space="DRAM") as dpool,
        ):
            # ---- constants ----
            iota_t = cpool.tile([P, P], bf16)
            nc.sync.dma_start(out=iota_t[:], in_=iota_in.ap())
            id_t = cpool.tile([P, P], f32)
            nc.sync.dma_start(out=id_t[:], in_=id_in.ap())
            dinv_t = cpool.tile([P, NW], f32)
            nc.sync.dma_start(out=dinv_t[:], in_=dinv_in.ap())
            w1_t = cpool.tile([P, KT, HID], bf16)
            nc.sync.dma_start(
                out=w1_t[:],
                in_=w1_in.ap().rearrange("(k p) h -> p k h", p=P))
            w2_t = cpool.tile([HID, HID], f32)
            nc.sync.dma_start(out=w2_t[:], in_=w2_in.ap())
            w3_t = cpool.tile([HID, OUT], f32)
            nc.sync.dma_start(out=w3_t[:], in_=w3_in.ap())
            b1_t = cpool.tile([P, HID], f32)
            nc.sync.dma_start(out=b1_t[:], in_=b1_in.ap())
            b2_t = cpool.tile([P, HID], f32)
            nc.sync.dma_start(out=b2_t[:], in_=b2_in.ap())
            b3_t = cpool.tile([P, OUT], f32)
            nc.sync.dma_start(out=b3_t[:], in_=b3_in.ap())
            zm_t = cpool.tile([P, 1], f32)
            nc.sync.dma_start(out=zm_t[:], in_=zmul_in.ap())
            dstl_t = cpool.tile([P, TOTC], bf16)
            nc.sync.dma_start(out=dstl_t[:], in_=dstl_in.ap())

            # ---- resident intermediates (shared across timing passes) ----
            t1_sb = cpool.tile([P, NW * HID], bf16)   # local table, layer 1
            t2_sb = cpool.tile([P, NW * HID], bf16)   # local table, layer 2
            h1_sb = cpool.tile([P, NW * HID], f32)    # hidden after layer 1

            # ---- carry chain helpers ----
            class Acc:
                """SBUF accumulator keeping every pass output live."""
                def __init__(self, tag):
                    self.t = hpool.tile([P, HID], f32, tag=f"acc_{tag}",
                                        name=f"acc_{tag}")
                    self.fresh = True

                def add(self, src_ap, width):
                    if self.fresh:
                        nc.gpsimd.memset(self.t[:], 0)
                        self.fresh = False
                    nc.vector.tensor_tensor(
                        out=self.t[:, :width], in0=self.t[:, :width],
                        in1=src_ap[:, :width], op=mybir.AluOpType.add)

                def store(self, carry_out):
                    nc.sync.dma_start(out=carry_out[:, :], in_=self.t[:])

            def load_caz(carry_in):
                ca = hpool.tile([P, HID], f32, tag="carry_rd")
                nc.sync.dma_start(out=ca[:], in_=carry_in[:, :])
                caz = hpool.tile([P, HID], f32, tag="carry_z")
                nc.vector.tensor_tensor(
                    out=caz[:], in0=ca[:],
                    in1=zm_t[:][:, :1].to_broadcast([P, HID]),
                    op=mybir.AluOpType.mult)
                return caz

            def acc_carry_in(acc, carry_in):
                if carry_in is None:
                    return
                assert acc is not None
                acc.add(load_caz(carry_in), HID)

            def inject_carry(carry_in, dst_ap, width):
                """dst_ap[:, :width] += zmul * carry (runtime no-op)."""
                caz = load_caz(carry_in)
                nc.vector.tensor_tensor(
                    out=dst_ap[:, :width], in0=dst_ap[:, :width],
                    in1=caz[:, :width], op=mybir.AluOpType.add)

            # ---- stages ----
            def transform1(tshq, collectives, carry_in=None, carry_out=None):
                """x @ W1 -> t1_sb + tshq[q]; fires collectives inline."""
                acc = Acc("t1") if carry_out is not None else None
                if acc is not None:
                    acc_carry_in(acc, carry_in)
                    carry_in = None
                BN = 512
                for b0 in range(0, NSH, BN):
                    bn = min(BN, NSH - b0)
                    tpsum = pscr.tile([HID, BN], f32, tag="scr")
                    for k in range(KT):
                        xk = xpool.tile([P, BN], bf16, tag="xk")
                        nc.sync.dma_start(
                            out=xk[:, :bn],
                            in_=x_t.ap()[k * P:(k + 1) * P, b0:b0 + bn])
                        nc.tensor.matmul(
                            tpsum[:, :bn], w1_t[:, k, :], xk[:, :bn],
                            start=(k == 0), stop=(k == KT - 1))
                    ts = ttpool.tile([HID, BN], f32, tag="ts")
                    nc.scalar.activation(ts[:, :bn], tpsum[:, :bn], AF.Copy)
                    for j in range(bn // P):
                        wdx = (b0 + j * P) // P
                        tp2 = pscr.tile([P, HID], f32, tag="scr")
                        nc.tensor.transpose(
                            tp2[:], ts[:, j * P:(j + 1) * P], id_t[:HID, :HID])
                        if carry_in is not None and wdx == 0:
                            inject_carry(carry_in, tp2, HID)
                        if acc is not None:
                            acc.add(tp2, HID)
                        nc.scalar.activation(
                            t1_sb[:, wdx * HID:(wdx + 1) * HID], tp2[:],
                            AF.Copy, scale=dinv_t[:, wdx:wdx + 1])
                        q = WQ[wdx]
                        r0 = (wdx - QW0[q]) * P
                        nc.sync.dma_start(
                            out=tshq[q][r0:r0 + P, :HID],
                            in_=t1_sb[:, wdx * HID:(wdx + 1) * HID])
                        if collectives is not None and wdx == q_last_w[q]:
                            collectives(q)
                if acc is not None:
                    acc.store(carry_out)

            def make_collectives(tshq, tblq):
                def fire(q):
                    nc.gpsimd.collective_compute(
                        "AllGather", mybir.AluOpType.bypass,
                        replica_groups=[list(range(NCORES))],
                        ins=[tshq[q].opt()], outs=[tblq[q].opt()])
                return fire

            def ag_standalone(tshq, tblq, carry_in=None, carry_out=None):
                if carry_in is not None:
                    ca = hpool.tile([P, HID], f32, tag="carry_rd")
                    nc.sync.dma_start(out=ca[:], in_=carry_in[:, :])
                    caz = hpool.tile([P, HID], f32, tag="carry_z")
                    nc.vector.tensor_tensor(
                        out=caz[:], in0=ca[:],
                        in1=zm_t[:][:, :1].to_broadcast([P, HID]),
                        op=mybir.AluOpType.mult)
                    cab = hpool.tile([P, HID], bf16, tag="carry_b")
                    nc.scalar.activation(cab[:], caz[:], AF.Copy)
                    nc.sync.dma_start(out=tshq[0][0:P, :HID], in_=cab[:])
                fire = make_collectives(tshq, tblq)
                for q in range(NQ):
                    fire(q)
                if carry_out is not None:
                    cr = hpool.tile([P, HID], bf16, tag="carry_rb")
                    nc.sync.dma_start(out=cr[:], in_=tblq[0][0:P, :HID])
                    crf = hpool.tile([P, HID], f32, tag="carry_rf")
                    nc.scalar.activation(crf[:], cr[:], AF.Copy)
                    nc.sync.dma_start(out=carry_out[:, :], in_=crf[:])

            # total chunk count per window (for matmul start/stop flags)
            wtot = {}
            for (gw0, gnw) in SWG:
                for wi in range(gnw):
                    wtot[gw0 + wi] = sum(granules[(gw0, q)]["wchunks"][wi]
                                         for q in range(NQ))

            def granule_gather(g, mode, gidx):
                """Issue the idx load + dma_gather for granule g; returns g3."""
                n_idx = g["n_idx"]
                nch = n_idx // P
                skip_g = ("nogather" in mode
                          or ("half" in mode and gidx % 2 == 1))
                gt = gpool.tile([P, nch * P], bf16, tag="gt")
                g3 = gt[:].rearrange("p (c d) -> p c d", d=P)
                if skip_g:
                    nc.gpsimd.memset(gt[:, :1], 0)
                    return gt, g3
                ix = ixpool.tile([P, n_idx // 16], i16, tag="ix")
                nc.sync.dma_start(
                    out=ix[:],
                    in_=idx_in.ap()[:, g["off"] // 16:(g["off"] + n_idx) // 16])
                nc.gpsimd.dma_gather(
                    out_ap=g3,
                    in_ap=tblq_cur[g["q"]][:, :],
                    idxs_ap=ix[:],
                    num_idxs=n_idx, num_idxs_reg=n_idx,
                    elem_size=P, elem_step=P,
                    single_packet=CFG["sp"] or ("sp" in mode),
                    queue_num=gidx % (2 if "q2" in mode else CFG["queues"]),
                )
                return gt, g3

            def evac_window(pss_wi, wdx, t_sb, bias_t, out_sb_or_head):
                """relu(dinv*(psum + t_local) + bias) -> h1_sb or head+out."""
                tcv = ttpool.tile([P, HID], f32, tag="tcv")
                nc.scalar.activation(
                    tcv[:], t_sb[:, wdx * HID:(wdx + 1) * HID], AF.Copy)
                ev1 = evpool.tile([P, HID], f32, tag="ev1")
                nc.vector.tensor_tensor(
                    out=ev1[:], in0=pss_wi[:], in1=tcv[:],
                    op=mybir.AluOpType.add)
                ev2 = evpool.tile([P, HID], f32, tag="ev2")
                nc.vector.scalar_tensor_tensor(
                    out=ev2[:], in0=ev1[:], scalar=dinv_t[:, wdx:wdx + 1],
                    in1=bias_t[:], op0=mybir.AluOpType.mult,
                    op1=mybir.AluOpType.add)
                return out_sb_or_head(ev2, wdx)

            def head_window(hw, wdx):
                """w3 head + log_softmax on one window; hw: [P, HID] f32."""
                hT_p = pscr.tile([HID, P], f32, tag="scr")
                nc.tensor.transpose(hT_p[:], hw[:], id_t[:])
                hT = ttpool.tile([HID, P], f32, tag="hTo")
                nc.scalar.activation(hT[:], hT_p[:], AF.Copy)
                lgT_p = pscr.tile([OUT, P], f32, tag="scr")
                nc.tensor.matmul(lgT_p[:], w3_t[:], hT[:], start=True, stop=True)
                lgT = ttpool.tile([OUT, P], f32, tag="lgT")
                nc.scalar.activation(lgT[:], lgT_p[:], AF.Copy)
                lg_p = pscr.tile([P, OUT], f32, tag="scr")
                nc.tensor.transpose(lg_p[:], lgT[:], id_t[:OUT, :OUT])
                lg = evpool.tile([P, OUT], f32, tag="lg")
                nc.vector.tensor_tensor(out=lg[:], in0=lg_p[:], in1=b3_t[:],
                                        op=mybir.AluOpType.add)
                nmax = evpool.tile([P, 1], f32, tag="nmax")
                nc.vector.reduce_max(nmax[:], lg[:], axis=mybir.AxisListType.X,
                                     negate=True)
                ex = evpool.tile([P, OUT], f32, tag="ex")
                nc.scalar.activation(ex[:], lg[:], AF.Exp, bias=nmax[:, :1])
                ssum = evpool.tile([P, 1], f32, tag="ssum")
                nc.vector.reduce_sum(ssum[:], ex[:], axis=mybir.AxisListType.X)
                lns = evpool.tile([P, 1], f32, tag="lns")
                nc.scalar.activation(lns[:], ssum[:], AF.Ln)
                tA = evpool.tile([P, OUT], f32, tag="tA")
                nc.vector.tensor_tensor(
                    out=tA[:], in0=lg[:],
                    in1=nmax[:][:, :1].to_broadcast([P, OUT]),
                    op=mybir.AluOpType.add)
                tB = evpool.tile([P, OUT], f32, tag="tB")
                nc.vector.tensor_tensor(
                    out=tB[:], in0=tA[:],
                    in1=lns[:][:, :1].to_broadcast([P, OUT]),
                    op=mybir.AluOpType.subtract)
                nc.sync.dma_start(out=out_ext.ap()[wdx * P:(wdx + 1) * P, :],
                                  in_=tB[:])
                return tB

            def transform2_window(wdx, tshq2, collectives2):
                hT_p = pscr.tile([HID, P], f32, tag="scr")
                nc.tensor.transpose(
                    hT_p[:], h1_sb[:, wdx * HID:(wdx + 1) * HID], id_t[:])
                hT = ttpool.tile([HID, P], f32, tag="hT")
                nc.scalar.activation(hT[:], hT_p[:], AF.Copy)
                t2T_p = pscr.tile([HID, P], f32, tag="scr")
                nc.tensor.matmul(t2T_p[:], w2_t[:], hT[:], start=True, stop=True)
                t2T = ttpool.tile([HID, P], f32, tag="t2T")
                nc.scalar.activation(t2T[:], t2T_p[:], AF.Copy)
                tp2 = pscr.tile([P, HID], f32, tag="scr")
                nc.tensor.transpose(tp2[:], t2T[:], id_t[:HID, :HID])
                nc.scalar.activation(
                    t2_sb[:, wdx * HID:(wdx + 1) * HID], tp2[:], AF.Copy,
                    scale=dinv_t[:, wdx:wdx + 1])
                q = WQ[wdx]
                r0 = (wdx - QW0[q]) * P
                nc.sync.dma_start(
                    out=tshq2[q][r0:r0 + P, :HID],
                    in_=t2_sb[:, wdx * HID:(wdx + 1) * HID])
                if collectives2 is not None and wdx == q_last_w[q]:
                    collectives2(q)
                return tp2

            def agg_layer(layer, t_sb, bias_t, with_t2, tshq2, coll2,
                          mode="", carry_in=None, carry_out=None):
                """One aggregation sweep; layer 1 writes h1_sb (+optional
                transform2 interleaved), layer 2 runs the fused head."""
                acc = Acc(f"agg{layer}") if carry_out is not None else None
                acc_carry_in(acc, carry_in)
                gidx = 0
                for (gw0, gnw) in SWG:
                    pss = [pagg.tile([P, HID], f32, tag="ps",
                                     name=f"ps{layer}_{gw0}_{wi}")
                           for wi in range(gnw)]
                    wseen = [0] * gnw
                    for q in range(NQ):
                        g = granules[(gw0, q)]
                        nch = g["n_idx"] // P
                        if nch == 0:
                            continue
                        gt, g3 = granule_gather(g, mode, gidx)
                        gidx += 1
                        if "nomm" in mode:
                            rs = evpool.tile([P, 1], f32, tag="rs")
                            nc.vector.reduce_sum(rs[:], gt[:],
                                                 axis=mybir.AxisListType.X)
                            acc.add(rs, 1)
                            continue
                        if "Sonly" in mode:
                            st = spool.tile([P, nch * P], bf16, tag="st")
                            s3 = st[:].rearrange("p (c q) -> p c q", q=P)
                            choff = g["off"] // P
                            nc.vector.tensor_tensor(
                                out=s3,
                                in0=dstl_t[:][:, choff:choff + nch, None]
                                    .to_broadcast([P, nch, P]),
                                in1=iota_t[:][:, None, :]
                                    .to_broadcast([P, nch, P]),
                                op=mybir.AluOpType.is_equal)
                            rs = evpool.tile([P, 1], f32, tag="rs")
                            nc.vector.reduce_sum(rs[:], gt[:],
                                                 axis=mybir.AxisListType.X)
                            acc.add(rs, 1)
                            rs2 = evpool.tile([P, 1], f32, tag="rs2")
                            nc.vector.reduce_sum(rs2[:], st[:],
                                                 axis=mybir.AxisListType.X)
                            acc.add(rs2, 1)
                            continue
                        if "noS" not in mode:
                            st = spool.tile([P, nch * P], bf16, tag="st")
                            s3 = st[:].rearrange("p (c q) -> p c q", q=P)
                            choff = g["off"] // P
                            nc.vector.tensor_tensor(
                                out=s3,
                                in0=dstl_t[:][:, choff:choff + nch, None]
                                    .to_broadcast([P, nch, P]),
                                in1=iota_t[:][:, None, :]
                                    .to_broadcast([P, nch, P]),
                                op=mybir.AluOpType.is_equal)
                        ck = 0
                        for wi in range(gnw):
                            ncw = g["wchunks"][wi]
                            tot = wtot[gw0 + wi]
                            for _k in range(ncw):
                                nc.tensor.matmul(
                                    pss[wi][:],
                                    iota_t[:] if "noS" in mode else s3[:, ck, :],
                                    g3[:, ck, :HID],
                                    start=(wseen[wi] == 0),
                                    stop=(wseen[wi] == tot - 1),
                                )
                                wseen[wi] += 1
                                ck += 1
                    if "nomm" in mode or "Sonly" in mode:
                        continue
                    for wi in range(gnw):
                        wdx = gw0 + wi
                        if layer == 1:
                            def sink(ev2, wdx=wdx):
                                nc.scalar.activation(
                                    h1_sb[:, wdx * HID:(wdx + 1) * HID],
                                    ev2[:], AF.Relu)
                                return None
                        else:
                            def sink(ev2, wdx=wdx):
                                hw = evpool.tile([P, HID], f32, tag="hw")
                                nc.scalar.activation(hw[:], ev2[:], AF.Relu)
                                return head_window(hw, wdx)
                        res = evac_window(pss[wi], wdx, t_sb, bias_t, sink)
                        if acc is not None:
                            if layer == 1:
                                acc.add(h1_sb[:, wdx * HID:(wdx + 1) * HID],
                                        HID)
                            elif res is not None:
                                acc.add(res, OUT)
                    if layer == 1 and with_t2:
                        for wi in range(gnw):
                            transform2_window(gw0 + wi, tshq2, coll2)
                if acc is not None:
                    acc.store(carry_out)

            def transform2_standalone(tshq2, coll2, carry_in=None,
                                      carry_out=None):
                acc = Acc("t2") if carry_out is not None else None
                acc_carry_in(acc, carry_in)
                for wdx in range(NW):
                    tp2 = transform2_window(wdx, tshq2, coll2)
                    if acc is not None:
                        acc.add(tp2, HID)
                if acc is not None:
                    acc.store(carry_out)

            # ---- passes ----
            all_stages = ("t1", "ag1", "agg1", "t2", "ag2", "agg2")
            carry_prev = None
            rep0_tblq = None
            for _rep in range(repeats):
                raw = all_stages if _rep == 0 else tuple(rep_stages)
                modes = {}
                st = []
                for s in raw:
                    name, _, m = s.partition(":")
                    st.append(name)
                    modes[name] = m
                st = tuple(st)
                r = f"r{_rep}"
                tshq1 = [dpool.tile([QC[q], P], bf16, name=f"tshq1_{q}{r}",
                                    tag=f"tshq1_{q}{r}") for q in range(NQ)]
                tshq2 = [dpool.tile([QC[q], P], bf16, name=f"tshq2_{q}{r}",
                                    tag=f"tshq2_{q}{r}") for q in range(NQ)]
                tblq1 = [dpool.tile([QROWS[q], P], bf16, addr_space="Shared",
                                    name=f"tblq1_{q}{r}", tag=f"tblq1_{q}{r}")
                         for q in range(NQ)]
                tblq2 = [dpool.tile([QROWS[q], P], bf16, addr_space="Shared",
                                    name=f"tblq2_{q}{r}", tag=f"tblq2_{q}{r}")
                         for q in range(NQ)]
                carry = (dpool.tile([P, HID], f32, name=f"carry{r}",
                                    tag=f"carry{r}")
                         if repeats > 1 else None)

                first = st[0]
                last = st[-1]

                def cio(name):
                    ci = carry_prev if (name == first and _rep > 0) else None
                    co = carry if (name == last and carry is not None) else None
                    return dict(carry_in=ci, carry_out=co)

                tblq1_src = tblq1 if "ag1" in st else rep0_tblq[0]
                tblq2_src = tblq2 if "ag2" in st else rep0_tblq[1]

                coll1 = make_collectives(tshq1, tblq1) if "ag1" in st else None
                coll2 = make_collectives(tshq2, tblq2) if "ag2" in st else None

                if "t1" in st:
                    transform1(tshq1, coll1, **cio("t1"))
                elif "ag1" in st:
                    ag_standalone(tshq1, tblq1, **cio("ag1"))
                if "agg1" in st:
                    global tblq_cur
                    tblq_cur = tblq1_src
                    agg_layer(1, t1_sb, b1_t, "t2" in st, tshq2, coll2,
                              mode=modes["agg1"], **cio("agg1"))
                elif "t2" in st:
                    transform2_standalone(tshq2, coll2, **cio("t2"))
                elif "ag2" in st:
                    ag_standalone(tshq2, tblq2, **cio("ag2"))
                if "agg2" in st:
                    tblq_cur = tblq2_src
                    agg_layer(2, t2_sb, b2_t, False, None, None,
                              mode=modes["agg2"], **cio("agg2"))

                if _rep == 0:
                    rep0_tblq = (tblq1, tblq2)
                carry_prev = carry

            if repeats > 1:
                # consume the final carry so no timing pass is dead code:
                # out[0:P] += zmul * carry  (zmul == 0 at runtime)
                caf = hpool.tile([P, HID], f32, tag="carry_fin")
                nc.sync.dma_start(out=caf[:], in_=carry_prev[:, :])
                cafz = hpool.tile([P, OUT], f32, tag="carry_finz")
                nc.vector.tensor_tensor(
                    out=cafz[:], in0=caf[:, :OUT],
                    in1=zm_t[:][:, :1].to_broadcast([P, OUT]),
                    op=mybir.AluOpType.mult)
                nc.gpsimd.dma_start(out=out_ext.ap()[0:P, :], in_=cafz[:],
                                    accum_op=mybir.AluOpType.add)

    nc.compile()
    return nc


def postprocess(meta, results):
    SH = meta["SH"]
    outs = [np.asarray(r["out"])[:SH] for r in results]
    return np.concatenate(outs, axis=0)


from concourse.bass_utils import run_bass_kernel_spmd


def kernel(x, edge_index, W1, b1, W2, b2, W3, b3):
    x = np.asarray(x)
    edge_index = np.asarray(edge_index)
    meta, in_maps = preprocess(x, edge_index, W1, b1, W2, b2, W3, b3)
    nc = build_program(meta)
    res = run_bass_kernel_spmd(nc, in_maps, list(range(NCORES)))
    out = postprocess(meta, res.results)
    return out.astype(np.float32)
